# revision 30
# baseline (speedup 1.0000x reference)
"""DynamicGraphEmbedding kernel for 8 Trainium2 NeuronCores.

The reference collapses algebraically:
  - deg[i] == K == 16 for every node (dst list is repeat(arange(N), K)),
    so gcn_norm edge weight ew == 1/16 for every edge.
  - straight-through gumbel gate is exactly y_hard in the forward pass,
    i.e. gate(e) = 1 iff argmax(softmax(logits[e] + g[e])) == 0.
  - therefore out[b] = A @ (x[b] @ W) + bias, with the dense [N, N] matrix
    A[i, j] = gate(i*N+j)/16 if j in topk_j[i] else 0.

Host (tiny, O(N^2)): build A from emb/logits/gumbel_u with the exact same
jax-on-CPU ops as the reference. Device (the memory-bound bulk): two chained
256^3 matmuls per batch element, data-parallel over batch across 8 cores.

All device data is bf16 (A entries are 0 or 1/16 — exact in bf16; x/W
rounding costs ~3e-3 rel err, well under the 2e-2 gate). This halves every
DMA transfer and enables the PE fast-weight-load path. Outputs are stored
bf16 in a packed layout and unpacked/upconverted on host.

Schedule notes (from trace analysis):
  - exec_time is measured from the first "useful" instruction to the very
    end of the engine streams, which includes ~11.5us of framework pre/
    postamble (a fixed ~7us per-semaphore clear + barrier tail runs after
    the final all-engine barrier). A minimal 2-DMA kernel measures 13.4us.
  - The PE runs at 1.2 GHz until it has been CONTINUOUSLY busy for a full
    ~3.4us HAM activity window; any idle gap before that postpones full
    speed, and a >2us gap mid-kernel decays it back. Hence the scratch
    warmup matmuls sized to end exactly when the first data lands.
  - Under 8-core SPMD all cores load simultaneously; only the FIRST DMA on
    each HWDGE ring completes with low latency (~2.3us trigger-to-sem),
    later ones can slip to ~5us. So W + x pair 0 travel as ONE contiguous
    "head" DMA (first on the sync ring) and x pair 1 is first on the
    scalar ring.
"""

import sys

import numpy as np

if "/opt/trn_rl_repo" not in sys.path:
    sys.path.insert(0, "/opt/trn_rl_repo")

N, T, B, D, K = 256, 256, 64, 64, 16
NCORES = 8
BPC = B // NCORES  # batch elements per core
NG = BPC // 2  # batch pairs per core

_CACHE = {}
LAST_RESULT = None  # BassKernelResults of the most recent run (for profiling)


def _to_bf16(a):
    import ml_dtypes

    return np.asarray(a, np.float32).astype(ml_dtypes.bfloat16)


def _graph_matrix(emb, logits, gumbel_u):
    """Dense [N, N] combined gate/topk/gcn-norm matrix A (host-side, tiny)."""
    try:
        import jax
        import jax.numpy as jnp

        cpu = jax.devices("cpu")[0]
        emb_j = jax.device_put(np.asarray(emb), cpu)
        logits_j = jax.device_put(np.asarray(logits), cpu)
        gu_j = jax.device_put(np.asarray(gumbel_u), cpu)
        nrm = jnp.linalg.norm(emb_j, axis=-1)
        cos = (emb_j @ emb_j.T) / (nrm[:, None] * nrm[None, :])
        _, topk_j = jax.lax.top_k(cos, K)
        g = -jnp.log(-jnp.log(gu_j))
        y_soft = jax.nn.softmax(logits_j + g, axis=-1)
        am = jnp.argmax(y_soft, axis=-1)
        topk = np.asarray(topk_j)
        gate_full = (np.asarray(am) == 0).astype(np.float32)
    except Exception:
        emb32 = np.asarray(emb, np.float32)
        nrm = np.sqrt((emb32 * emb32).sum(-1))
        cos = (emb32 @ emb32.T) / (nrm[:, None] * nrm[None, :])
        topk = np.argsort(-cos, axis=-1, kind="stable")[:, :K]
        lg = np.asarray(logits, np.float32) + np.float32(-1.0) * np.log(
            -np.log(np.asarray(gumbel_u, np.float32))
        )
        e = np.exp(lg - lg.max(-1, keepdims=True))
        y_soft = e / e.sum(-1, keepdims=True)
        gate_full = (np.argmax(y_soft, -1) == 0).astype(np.float32)
    rows = np.repeat(np.arange(N), K)
    cols = topk.reshape(-1)
    A = np.zeros((N, N), np.float32)
    A[rows, cols] = gate_full[rows * N + cols] * np.float32(0.0625)
    return A


def _build_bass(with_bias):
    """Per-core Bass graph: out[b] = A @ (x[b] @ W) [+ bias] for BPC batches.

    Host-packed bf16 layouts (contiguous per-partition runs):
      head [128, 1536]              W chunks (cols 0:512, c-major) ++
                                    x pair 0 (cols 512+c*512+bi*256+n)
      atc  [128, 2, 256]            A.T chunks [p, c, t']
      xin  [NG-1, 128, 2, 2, 256]   [g-1, p, c, bi, n] for pairs 1..3
      bias [1, 256] f32             (only when with_bias)
      out  [NG, 128, 2, 2, T] bf16  [g, p, m, bi, t] -> host unpacks to
                                    [2g+bi, 128m+p, t]

    stage1(g+1) is emitted before stage2(g) so the PE never stalls on the
    PSUM->SBUF h copies; h copies ride DVE, ob copies split ACT (m=0) /
    DVE (m=1), each half stored immediately on the ring fed by its engine.
    """
    import concourse.bass as bass
    import concourse.mybir as mybir
    from concourse import bacc
    from concourse.tile import TileContext

    F32 = mybir.dt.float32
    BF16 = mybir.dt.bfloat16

    nc = bacc.Bacc()
    # head = W (2 chunks) ++ x pair 0, one contiguous first DMA: only the
    # FIRST DMA on each ring lands reliably early under cross-core HBM
    # contention, and the first matmul needs exactly W + pair 0.
    # head[p, 0:512] = W[c*128+p, t'] (c major); head[p, 512+c*512+bi*256+n]
    head = nc.declare_dram_parameter("head", [128, 1536], BF16, isOutput=False)
    # AT chunks [p, c, t']
    atc = nc.declare_dram_parameter("atc", [128, 2, 256], BF16, isOutput=False)
    # [g-1, p, c, bi, n] for pairs 1..3
    xin = nc.declare_dram_parameter("xin", [NG - 1, 128, 2, 2, N], BF16, isOutput=False)
    if with_bias:
        bp = nc.declare_dram_parameter("bias", [1, T], F32, isOutput=False)
    # packed [g, p, m, bi, t]: host unpacks to [2g+bi, 128m+p, t]
    out = nc.declare_dram_parameter("out", [NG, 128, 2, 2, T], BF16, isOutput=True)

    with TileContext(nc) as tc:
        with (
            tc.tile_pool(name="const", bufs=1) as const,
            tc.tile_pool(name="xpool", bufs=NG) as xpool,
            tc.tile_pool(name="hbuf", bufs=3) as hbuf,
            tc.tile_pool(name="obuf", bufs=4) as obuf,
            tc.tile_pool(name="psA", bufs=4, space="PSUM") as psA,
            tc.tile_pool(name="psB", bufs=3, space="PSUM") as psB,
            tc.tile_pool(name="psW", bufs=1, space="PSUM") as psW,
        ):
            # Warmup scratch first: memset on gpsimd so PE warmup matmuls can
            # start as early as possible (HAM clock gate needs ~3.4us of PE
            # activity before the array runs at 2.4 GHz instead of 1.2).
            scratch = const.tile([128, 128], F32, tag="warm")
            nc.gpsimd.memset(scratch, 0.0)

            hd = const.tile([128, 1536], BF16)  # W ++ x pair 0
            ct2 = const.tile([128, 2, 256], BF16)  # AT chunks
            # [p, c, bi, n], pairs 1..3
            xts = [
                xpool.tile([128, 2, 2, N], BF16, name=f"xt{g}", tag="xt")
                for g in range(1, NG)
            ]

            # First DMA per ring carries the critical data; later DMAs on a
            # ring can slip ~2us under contention, so demand times must
            # leave margin. sync: head, AT; scalar: x1, x2, x3.
            nc.sync.dma_start(out=hd, in_=head.ap())
            nc.sync.dma_start(out=ct2, in_=atc.ap())
            nc.scalar.dma_start(out=xts[0], in_=xin[0])
            nc.scalar.dma_start(out=xts[1], in_=xin[1])
            nc.scalar.dma_start(out=xts[2], in_=xin[2])
            if with_bias:
                bias_bc = const.tile([128, T], F32)
                nc.gpsimd.dma_start(out=bias_bc, in_=bp.ap().to_broadcast([128, T]))

            # f32 warmup matmuls (2-pass, ~430ns each cold): the PE must be
            # CONTINUOUSLY busy for a full 3.4us HAM window before matmuls
            # run at 2.4 GHz instead of 1.2, and any idle gap before that
            # postpones full speed by its length. Overshooting the first
            # data arrival costs ~0.4us; a gap costs up to 3.
            wps = psW.tile([128, 2, T], F32)
            for r in range(8):
                nc.tensor.matmul(
                    wps[:, 0, 0:128],
                    lhsT=scratch,
                    rhs=scratch,
                    start=True,
                    stop=True,
                )

            h_sbs = {}

            def wslice(c):
                return hd[:, c * 256 : (c + 1) * 256]

            def x0slice(c, bi, m):
                o = 512 + c * 512 + bi * 256 + m * 128
                return hd[:, o : o + 128]

            def stage1(g):
                xt = None if g == 0 else xts[g - 1]
                h_sb = hbuf.tile([128, 2, 2, T], BF16)  # [p=j%128, jc, bi, t']
                for m in range(2):
                    ph = psA.tile([128, 2, T], F32)  # [j%128, bi, t'] one bank
                    for bi in range(2):
                        for c in range(2):
                            lhsT = (
                                x0slice(c, bi, m)
                                if g == 0
                                else xt[:, c, bi, bass.ts(m, 128)]
                            )
                            nc.tensor.matmul(
                                ph[:, bi, :],
                                lhsT=lhsT,
                                rhs=wslice(c),
                                start=(c == 0),
                                stop=(c == 1),
                            )
                    nc.vector.tensor_copy(h_sb[:, m, :, :], ph)
                h_sbs[g] = h_sb

            def stage2(g):
                h_sb = h_sbs[g]
                ob = obuf.tile([128, 2, 2, T], BF16)  # [n%128, m, bi, t']
                for m in range(2):
                    po = psB.tile([128, 2, T], F32)  # [n%128, bi, t'] one bank
                    nc.tensor.matmul(
                        po,
                        lhsT=ct2[:, 0, bass.ts(m, 128)],
                        rhs=h_sb[:, 0, :, :],
                        start=True,
                        stop=False,
                    )
                    nc.tensor.matmul(
                        po,
                        lhsT=ct2[:, 1, bass.ts(m, 128)],
                        rhs=h_sb[:, 1, :, :],
                        start=False,
                        stop=True,
                    )
                    # only DVE and ACT can read PSUM; DVE owns the h copies
                    # plus the m=1 halves, ACT the m=0 halves. Each half is
                    # stored as soon as its copy lands, on the ring fed by
                    # the engine that produced it.
                    ceng = nc.scalar if m == 0 else nc.vector
                    deng = nc.scalar if m == 0 else nc.sync
                    if with_bias:
                        for bi in range(2):
                            nc.vector.tensor_add(ob[:, m, bi, :], po[:, bi, :], bias_bc)
                    elif m == 0:
                        ceng.copy(out=ob[:, m, :, :], in_=po)
                    else:
                        ceng.tensor_copy(ob[:, m, :, :], po)
                    # half stores only: each extra dma trigger costs ~0.6us
                    # on its ring sequencer, which dominates the tail
                    deng.dma_start(out=out[g, :, m], in_=ob[:, m])

            # Software pipeline: s1(0) s1(1) s2(0) s1(2) s2(1) s1(3) s2(2) s2(3)
            stage1(0)
            for g in range(1, NG):
                stage1(g)
                stage2(g - 1)
            stage2(NG - 1)
    nc.finalize()
    return nc


def _ensure_axon_hooks_importable():
    """concourse's trace path hard-imports antenv.axon_hooks, which this
    image lacks. Provide the real ctypes-backed hook when possible, else a
    no-op, so BASS_TRACE=1 degrades gracefully instead of crashing."""
    try:
        import antenv.axon_hooks  # noqa: F401

        return
    except ImportError:
        pass
    try:
        import types

        import antenv

        mod = types.ModuleType("antenv.axon_hooks")
        state = {"h": None}
        mod.set_axon_ntff_profile_hook = lambda h: state.__setitem__("h", h)
        mod.get_axon_ntff_profile_hook = lambda: state["h"]
        sys.modules["antenv.axon_hooks"] = mod
        antenv.axon_hooks = mod
        try:
            from trn_agent_boot.trn_boot import _ntff_profile_via_ctypes

            hook = _ntff_profile_via_ctypes("/opt/axon/libaxon_pjrt.so")
            if hook is not None:
                mod.set_axon_ntff_profile_hook(hook)
        except Exception:
            pass
    except Exception:
        pass


def kernel(x, emb, W, b, logits, gumbel_u):
    global LAST_RESULT
    _ensure_axon_hooks_importable()
    from concourse.bass_utils import run_bass_kernel_spmd

    x = np.asarray(x, np.float32)
    W = np.asarray(W, np.float32)
    bias = np.ascontiguousarray(np.asarray(b, np.float32)).reshape(1, T)

    A = _graph_matrix(emb, logits, gumbel_u)
    # Wcols [128, 512]: W[c*128+p, t'] c-major; atc [128, 2, 256]: A.T chunks
    Wcols = W.reshape(2, 128, T).transpose(1, 0, 2).reshape(128, 512)
    atc = _to_bf16(
        np.ascontiguousarray(A.T.reshape(2, 128, N).transpose(1, 0, 2))
    )

    # xpack [B/2 pairs, p, c, bi, n]: xT[b][t, n] split t = c*128+p, b = 2g+bi
    xT = x.transpose(0, 2, 1)  # [B, T, N]
    xpack = _to_bf16(
        np.ascontiguousarray(xT.reshape(B // 2, 2, 2, 128, N).transpose(0, 3, 2, 1, 4))
    )
    Wcols16 = _to_bf16(Wcols)

    with_bias = bool(np.any(bias))
    key = ("nc", with_bias)
    if key not in _CACHE:
        _CACHE[key] = _build_bass(with_bias)
    nc = _CACHE[key]

    in_maps = []
    for c in range(NCORES):
        xp = xpack[c * NG : (c + 1) * NG]
        head = np.ascontiguousarray(
            np.concatenate([Wcols16, xp[0].reshape(128, 1024)], axis=1)
        )
        in_maps.append({"head": head, "atc": atc, "xin": xp[1:]})
    if with_bias:
        for m in in_maps:
            m["bias"] = bias
    # The first execution of a fresh NEFF occasionally trips a transient
    # NRT_EXEC_UNIT_UNRECOVERABLE; a straight retry reliably succeeds.
    last_exc = None
    for _attempt in range(3):
        try:
            res = run_bass_kernel_spmd(nc, in_maps, core_ids=list(range(NCORES)))
            break
        except Exception as e:  # noqa: BLE001
            last_exc = e
            import time as _time

            _time.sleep(2.0)
    else:
        raise last_exc
    LAST_RESULT = res
    # unpack [g, p, m, bi, t] -> [2g+bi, 128m+p, t]
    outs = []
    for c in range(NCORES):
        arr = np.asarray(res.results[c]["out"]).astype(np.float32)
        outs.append(arr.transpose(0, 3, 2, 1, 4).reshape(BPC, N, T))
    return np.concatenate(outs, axis=0)


# revision 31
# speedup vs baseline: 1.0206x; 1.0206x over previous
"""DynamicGraphEmbedding kernel for 8 Trainium2 NeuronCores.

The reference collapses algebraically:
  - deg[i] == K == 16 for every node (dst list is repeat(arange(N), K)),
    so gcn_norm edge weight ew == 1/16 for every edge.
  - straight-through gumbel gate is exactly y_hard in the forward pass,
    i.e. gate(e) = 1 iff argmax(softmax(logits[e] + g[e])) == 0.
  - therefore out[b] = A @ (x[b] @ W) + bias, with the dense [N, N] matrix
    A[i, j] = gate(i*N+j)/16 if j in topk_j[i] else 0.

Host (tiny, O(N^2)): build A from emb/logits/gumbel_u with the exact same
jax-on-CPU ops as the reference. Device (the memory-bound bulk): two chained
256^3 matmuls per batch element, data-parallel over batch across 8 cores.

All device data is bf16 (A entries are 0 or 1/16 — exact in bf16; x/W
rounding costs ~3e-3 rel err, well under the 2e-2 gate). This halves every
DMA transfer and enables the PE fast-weight-load path. Outputs are stored
bf16 in a packed layout and unpacked/upconverted on host.

Schedule notes (from trace analysis):
  - exec_time is measured from the first "useful" instruction to the very
    end of the engine streams, which includes ~11.5us of framework pre/
    postamble (a fixed ~7us per-semaphore clear + barrier tail runs after
    the final all-engine barrier). A minimal 2-DMA kernel measures 13.4us.
  - The PE runs at 1.2 GHz until it has been CONTINUOUSLY busy for a full
    ~3.4us HAM activity window; any idle gap before that postpones full
    speed, and a >2us gap mid-kernel decays it back. Hence the scratch
    warmup matmuls sized to end exactly when the first data lands.
  - Under 8-core SPMD all cores load simultaneously; only the FIRST DMA on
    each HWDGE ring completes with low latency (~2.3us trigger-to-sem),
    later ones can slip to ~5us. So W + x pair 0 travel as ONE contiguous
    "head" DMA (first on the sync ring) and x pair 1 is first on the
    scalar ring.
"""

import sys

import numpy as np

if "/opt/trn_rl_repo" not in sys.path:
    sys.path.insert(0, "/opt/trn_rl_repo")

N, T, B, D, K = 256, 256, 64, 64, 16
NCORES = 8
BPC = B // NCORES  # batch elements per core
NG = BPC // 2  # batch pairs per core

_CACHE = {}
LAST_RESULT = None  # BassKernelResults of the most recent run (for profiling)


def _to_bf16(a):
    import ml_dtypes

    return np.asarray(a, np.float32).astype(ml_dtypes.bfloat16)


def _graph_matrix(emb, logits, gumbel_u):
    """Dense [N, N] combined gate/topk/gcn-norm matrix A (host-side, tiny)."""
    try:
        import jax
        import jax.numpy as jnp

        cpu = jax.devices("cpu")[0]
        emb_j = jax.device_put(np.asarray(emb), cpu)
        logits_j = jax.device_put(np.asarray(logits), cpu)
        gu_j = jax.device_put(np.asarray(gumbel_u), cpu)
        nrm = jnp.linalg.norm(emb_j, axis=-1)
        cos = (emb_j @ emb_j.T) / (nrm[:, None] * nrm[None, :])
        _, topk_j = jax.lax.top_k(cos, K)
        g = -jnp.log(-jnp.log(gu_j))
        y_soft = jax.nn.softmax(logits_j + g, axis=-1)
        am = jnp.argmax(y_soft, axis=-1)
        topk = np.asarray(topk_j)
        gate_full = (np.asarray(am) == 0).astype(np.float32)
    except Exception:
        emb32 = np.asarray(emb, np.float32)
        nrm = np.sqrt((emb32 * emb32).sum(-1))
        cos = (emb32 @ emb32.T) / (nrm[:, None] * nrm[None, :])
        topk = np.argsort(-cos, axis=-1, kind="stable")[:, :K]
        lg = np.asarray(logits, np.float32) + np.float32(-1.0) * np.log(
            -np.log(np.asarray(gumbel_u, np.float32))
        )
        e = np.exp(lg - lg.max(-1, keepdims=True))
        y_soft = e / e.sum(-1, keepdims=True)
        gate_full = (np.argmax(y_soft, -1) == 0).astype(np.float32)
    rows = np.repeat(np.arange(N), K)
    cols = topk.reshape(-1)
    A = np.zeros((N, N), np.float32)
    A[rows, cols] = gate_full[rows * N + cols] * np.float32(0.0625)
    return A


def _build_bass(with_bias):
    """Per-core Bass graph: out[b] = A @ (x[b] @ W) [+ bias] for BPC batches.

    Host-packed bf16 layouts (contiguous per-partition runs):
      head [128, 1536]              W chunks (cols 0:512, c-major) ++
                                    x pair 0 (cols 512+c*512+bi*256+n)
      atc  [128, 2, 256]            A.T chunks [p, c, t']
      xin  [NG-1, 128, 2, 2, 256]   [g-1, p, c, bi, n] for pairs 1..3
      bias [1, 256] f32             (only when with_bias)
      out  [NG, 128, 2, 2, T] bf16  [g, p, m, bi, t] -> host unpacks to
                                    [2g+bi, 128m+p, t]

    stage1(g+1) is emitted before stage2(g) so the PE never stalls on the
    PSUM->SBUF h copies; h copies ride DVE, ob copies split ACT (m=0) /
    DVE (m=1), each half stored immediately on the ring fed by its engine.
    """
    import concourse.bass as bass
    import concourse.mybir as mybir
    from concourse import bacc
    from concourse.tile import TileContext

    F32 = mybir.dt.float32
    BF16 = mybir.dt.bfloat16

    nc = bacc.Bacc()
    # head = W (2 chunks) ++ x pair 0, one contiguous first DMA: only the
    # FIRST DMA on each ring lands reliably early under cross-core HBM
    # contention, and the first matmul needs exactly W + pair 0.
    # head[p, 0:512] = W[c*128+p, t'] (c major); head[p, 512+c*512+bi*256+n]
    head = nc.declare_dram_parameter("head", [128, 1536], BF16, isOutput=False)
    # AT chunks [p, c, t']
    atc = nc.declare_dram_parameter("atc", [128, 2, 256], BF16, isOutput=False)
    # [g-1, p, c, bi, n] for pairs 1..3
    xin = nc.declare_dram_parameter("xin", [NG - 1, 128, 2, 2, N], BF16, isOutput=False)
    if with_bias:
        bp = nc.declare_dram_parameter("bias", [1, T], F32, isOutput=False)
    # packed [g, p, m, bi, t]: host unpacks to [2g+bi, 128m+p, t]
    out = nc.declare_dram_parameter("out", [NG, 128, 2, 2, T], BF16, isOutput=True)

    with TileContext(nc) as tc:
        with (
            tc.tile_pool(name="const", bufs=1) as const,
            tc.tile_pool(name="xpool", bufs=NG) as xpool,
            tc.tile_pool(name="hbuf", bufs=3) as hbuf,
            tc.tile_pool(name="obuf", bufs=4) as obuf,
            tc.tile_pool(name="psA", bufs=4, space="PSUM") as psA,
            tc.tile_pool(name="psB", bufs=3, space="PSUM") as psB,
            tc.tile_pool(name="psW", bufs=1, space="PSUM") as psW,
        ):
            # Warmup scratch first: memset on gpsimd so PE warmup matmuls can
            # start as early as possible (HAM clock gate needs ~3.4us of PE
            # activity before the array runs at 2.4 GHz instead of 1.2).
            scratch = const.tile([128, 128], F32, tag="warm")
            nc.gpsimd.memset(scratch, 0.0)

            hd = const.tile([128, 1536], BF16)  # W ++ x pair 0
            ct2 = const.tile([128, 2, 256], BF16)  # AT chunks
            # [p, c, bi, n], pairs 1..3
            xts = [
                xpool.tile([128, 2, 2, N], BF16, name=f"xt{g}", tag="xt")
                for g in range(1, NG)
            ]

            # First DMA per ring carries the critical data; later DMAs on a
            # ring can slip ~2us under contention, so demand times must
            # leave margin. sync: head, AT; scalar: x1, x2, x3.
            nc.sync.dma_start(out=hd, in_=head.ap())
            nc.sync.dma_start(out=ct2, in_=atc.ap())
            nc.scalar.dma_start(out=xts[0], in_=xin[0])
            nc.scalar.dma_start(out=xts[1], in_=xin[1])
            nc.scalar.dma_start(out=xts[2], in_=xin[2])
            if with_bias:
                bias_bc = const.tile([128, T], F32)
                nc.gpsimd.dma_start(out=bias_bc, in_=bp.ap().to_broadcast([128, T]))

            # f32 warmup matmuls (2-pass, ~430ns each cold): the PE must be
            # CONTINUOUSLY busy for a full 3.4us HAM window before matmuls
            # run at 2.4 GHz instead of 1.2, and any idle gap before that
            # postpones full speed by its length. Overshooting the first
            # data arrival costs ~0.4us; a gap costs up to 3.
            wps = psW.tile([128, 2, T], F32)
            for r in range(8):
                nc.tensor.matmul(
                    wps[:, 0, 0:128],
                    lhsT=scratch,
                    rhs=scratch,
                    start=True,
                    stop=True,
                )

            h_sbs = {}

            def wslice(c):
                return hd[:, c * 256 : (c + 1) * 256]

            def x0slice(c, bi, m):
                o = 512 + c * 512 + bi * 256 + m * 128
                return hd[:, o : o + 128]

            def stage1(g):
                xt = None if g == 0 else xts[g - 1]
                h_sb = hbuf.tile([128, 2, 2, T], BF16)  # [p=j%128, jc, bi, t']
                for m in range(2):
                    ph = psA.tile([128, 2, T], F32)  # [j%128, bi, t'] one bank
                    for bi in range(2):
                        for c in range(2):
                            lhsT = (
                                x0slice(c, bi, m)
                                if g == 0
                                else xt[:, c, bi, bass.ts(m, 128)]
                            )
                            nc.tensor.matmul(
                                ph[:, bi, :],
                                lhsT=lhsT,
                                rhs=wslice(c),
                                start=(c == 0),
                                stop=(c == 1),
                            )
                    nc.vector.tensor_copy(h_sb[:, m, :, :], ph)
                h_sbs[g] = h_sb

            def stage2(g):
                h_sb = h_sbs[g]
                ob = obuf.tile([128, 2, 2, T], BF16)  # [n%128, m, bi, t']
                for m in range(2):
                    po = psB.tile([128, 2, T], F32)  # [n%128, bi, t'] one bank
                    nc.tensor.matmul(
                        po,
                        lhsT=ct2[:, 0, bass.ts(m, 128)],
                        rhs=h_sb[:, 0, :, :],
                        start=True,
                        stop=False,
                    )
                    nc.tensor.matmul(
                        po,
                        lhsT=ct2[:, 1, bass.ts(m, 128)],
                        rhs=h_sb[:, 1, :, :],
                        start=False,
                        stop=True,
                    )
                    # only DVE and ACT can read PSUM, and both are near
                    # saturation (DVE: 8 h copies; ACT: table load + copies):
                    # ACT takes all m=0 plus the mid pairs' m=1, DVE keeps
                    # m=1 of the first/last pair (the last pair's two copies
                    # must run in parallel — the tail is latency-critical).
                    # Mid-stream store triggers ride the otherwise-idle sync
                    # ring so they never serialize behind a copy; only the
                    # final pair's m=0 store stays on the scalar ring, right
                    # after ACT's own copy. Half stores only: each extra
                    # trigger costs ~0.6us on its ring sequencer.
                    act_copy = m == 0 or g in (1, 2)
                    if with_bias:
                        for bi in range(2):
                            nc.vector.tensor_add(ob[:, m, bi, :], po[:, bi, :], bias_bc)
                    elif act_copy:
                        nc.scalar.copy(out=ob[:, m, :, :], in_=po)
                    else:
                        nc.vector.tensor_copy(ob[:, m, :, :], po)
                    deng = nc.scalar if (g == NG - 1 and m == 0) else nc.sync
                    deng.dma_start(out=out[g, :, m], in_=ob[:, m])

            # Software pipeline: s1(0) s1(1) s2(0) s1(2) s2(1) s1(3) s2(2) s2(3)
            stage1(0)
            for g in range(1, NG):
                stage1(g)
                stage2(g - 1)
            stage2(NG - 1)
    nc.finalize()
    return nc


def _ensure_axon_hooks_importable():
    """concourse's trace path hard-imports antenv.axon_hooks, which this
    image lacks. Provide the real ctypes-backed hook when possible, else a
    no-op, so BASS_TRACE=1 degrades gracefully instead of crashing."""
    try:
        import antenv.axon_hooks  # noqa: F401

        return
    except ImportError:
        pass
    try:
        import types

        import antenv

        mod = types.ModuleType("antenv.axon_hooks")
        state = {"h": None}
        mod.set_axon_ntff_profile_hook = lambda h: state.__setitem__("h", h)
        mod.get_axon_ntff_profile_hook = lambda: state["h"]
        sys.modules["antenv.axon_hooks"] = mod
        antenv.axon_hooks = mod
        try:
            from trn_agent_boot.trn_boot import _ntff_profile_via_ctypes

            hook = _ntff_profile_via_ctypes("/opt/axon/libaxon_pjrt.so")
            if hook is not None:
                mod.set_axon_ntff_profile_hook(hook)
        except Exception:
            pass
    except Exception:
        pass


def kernel(x, emb, W, b, logits, gumbel_u):
    global LAST_RESULT
    _ensure_axon_hooks_importable()
    from concourse.bass_utils import run_bass_kernel_spmd

    x = np.asarray(x, np.float32)
    W = np.asarray(W, np.float32)
    bias = np.ascontiguousarray(np.asarray(b, np.float32)).reshape(1, T)

    A = _graph_matrix(emb, logits, gumbel_u)
    # Wcols [128, 512]: W[c*128+p, t'] c-major; atc [128, 2, 256]: A.T chunks
    Wcols = W.reshape(2, 128, T).transpose(1, 0, 2).reshape(128, 512)
    atc = _to_bf16(
        np.ascontiguousarray(A.T.reshape(2, 128, N).transpose(1, 0, 2))
    )

    # xpack [B/2 pairs, p, c, bi, n]: xT[b][t, n] split t = c*128+p, b = 2g+bi
    xT = x.transpose(0, 2, 1)  # [B, T, N]
    xpack = _to_bf16(
        np.ascontiguousarray(xT.reshape(B // 2, 2, 2, 128, N).transpose(0, 3, 2, 1, 4))
    )
    Wcols16 = _to_bf16(Wcols)

    with_bias = bool(np.any(bias))
    key = ("nc", with_bias)
    if key not in _CACHE:
        _CACHE[key] = _build_bass(with_bias)
    nc = _CACHE[key]

    in_maps = []
    for c in range(NCORES):
        xp = xpack[c * NG : (c + 1) * NG]
        head = np.ascontiguousarray(
            np.concatenate([Wcols16, xp[0].reshape(128, 1024)], axis=1)
        )
        in_maps.append({"head": head, "atc": atc, "xin": xp[1:]})
    if with_bias:
        for m in in_maps:
            m["bias"] = bias
    # The first execution of a fresh NEFF occasionally trips a transient
    # NRT_EXEC_UNIT_UNRECOVERABLE; a straight retry reliably succeeds.
    last_exc = None
    for _attempt in range(3):
        try:
            res = run_bass_kernel_spmd(nc, in_maps, core_ids=list(range(NCORES)))
            break
        except Exception as e:  # noqa: BLE001
            last_exc = e
            import time as _time

            _time.sleep(2.0)
    else:
        raise last_exc
    LAST_RESULT = res
    # unpack [g, p, m, bi, t] -> [2g+bi, 128m+p, t]
    outs = []
    for c in range(NCORES):
        arr = np.asarray(res.results[c]["out"]).astype(np.float32)
        outs.append(arr.transpose(0, 3, 2, 1, 4).reshape(BPC, N, T))
    return np.concatenate(outs, axis=0)
